# revision 12
# baseline (speedup 1.0000x reference)
"""Trainium2 Bass kernel for Conv2D(sum of 20 1x1 convs) + QwenRMSNorm.

Math: y = einsum("bsi,loi->bso", x, conv_w) / L ; out = rmsnorm(y) * norm_w.
Since x does not depend on l, the 20-matrix contraction collapses to a single
matmul with W = sum_l conv_w[l] / L.  Host pre-sums/transposes/casts the weight
(one [H,H] matrix) and lays out x as token-sharded, hidden-major bf16 slabs;
the 8 NeuronCores each run matmul (bf16, fp32 accum) + RMSNorm on their 2048
tokens.  All device compute is token-local; no collectives.

Performance structure (from NTFF trace analysis):
 - exec time ~= (last output-DMA completion) + fixed ~8us engine-drain
   epilogue - ~6.4us preamble; the window starts at the framework's own
   constant memsets, so only data-start (~8.5us) onwards is controllable.
 - HBM bandwidth (~358 GB/s/core) is split evenly over ACTIVE queue rows
   (sync=qSP HWDGE, scalar=qAct HWDGE, gpsimd=SWDGE), FIFO within a row,
   no cross-row priority -> every row's FIFO must be ordered by consumption
   time, and bulk x transfers must sit BEHIND the 2 MiB weight load.
 - ~1.7us completion-semaphore receipt after each DMA's last byte; 8 global
   HWDGE semaphore lanes -> few, large DMAs.
 - PE HAM clock gate: 1.2GHz until ~3.4us of sustained busy; dummy matmuls
   bridge the preamble to the first weight arrival so real matmuls run at
   2.4GHz; the first three token blocks interleave ib-outer so each weight
   chunk arrival unlocks 6 matmuls immediately.
 - RMSNorm: per-half [128,512] psum tiles; squares+variance on ACT with the
   hardware accumulator; when norm_w == 1 (the spec fill) the final
   y*rstd multiply runs as ACT Copy(scale=rstd) for one half concurrently
   with DVE (in0*scalar, op1=bypass) for the other, and the norm_w load is
   skipped entirely; a general norm_w path is kept as fallback.
"""

import numpy as np
import ml_dtypes
from contextlib import ExitStack

import concourse.bass as bass
import concourse.mybir as mybir
import concourse.tile as tile
from concourse.bass_utils import run_bass_kernel_spmd

N_CORES = 8
B, S, H, L = 4, 4096, 1024, 20
TOK = B * S               # 16384 tokens
TPC = TOK // N_CORES      # 2048 tokens per core
TB = TPC // 128           # 16 token-blocks of 128 per core
KB = H // 128             # 8 contraction blocks
EPS = 1e-6
N_WARM = 10               # pre-warm matmuls bridging preamble -> first data

BF16 = mybir.dt.bfloat16
F32 = mybir.dt.float32
AF = mybir.ActivationFunctionType
OP = mybir.AluOpType

_BUILT = {}          # cached Bass programs keyed by nw_ones
LAST_RESULTS = None  # BassKernelResults of the most recent run (for test harness)


def _legalize_multiwait(nc):
    """The walrus build here encodes exactly one semaphore wait per 64B
    instruction (NEURON_ISA_TPB_EVENTS has a single wait slot) and errors on
    Tile's multi-wait instructions.  Split surplus waits into standalone
    EVENT_SEMAPHORE instructions on the same engine, placed directly before
    the original instruction (same sequencer stream -> same semantics)."""
    n_ev = 0
    for f in nc.m.functions:
        for blk in f.blocks:
            insts = blk.instructions
            out = []
            changed = False
            for inst in list(insts):
                si = getattr(inst, "sync_info", None)
                waits = list(si.on_wait) if si is not None else []
                if len(waits) > 1:
                    changed = True
                    updates = list(si.on_update)
                    for w in waits[:-1]:
                        ev = mybir.InstEventSemaphore(
                            name=f"{inst.name}-sw{n_ev}", ins=[], outs=[])
                        n_ev += 1
                        ev.engine = inst.engine
                        ev.sync_info = mybir.SyncInfo(on_wait=[w], on_update=[])
                        out.append(ev)
                    inst.sync_info = mybir.SyncInfo(
                        on_wait=[waits[-1]], on_update=updates)
                out.append(inst)
            if changed:
                insts.clear()
                insts.extend(out)


def _build(nw_ones):
    nc = bass.Bass()
    # x^T slab layout per core: xt[p, tt, ib, t] = x[tt*128 + t, ib*128 + p],
    # bf16, partition-major so multi-slab group DMAs keep matching AP order
    xt_h = nc.dram_tensor("xt", [128, TB, KB, 128], BF16, kind="ExternalInput")
    # weight layout: wt[p, ib, o] = W[o, ib*128 + p] with W = sum_l conv_w[l]/L
    wt_h = nc.dram_tensor("wt", [128, KB, H], BF16, kind="ExternalInput")
    nw_h = nc.dram_tensor("nw", [128, H], F32, kind="ExternalInput")
    out_h = nc.dram_tensor("out", [TPC, H], BF16, kind="ExternalOutput")

    with tile.TileContext(nc) as tc, ExitStack() as ctx:
        xpool = ctx.enter_context(tc.tile_pool(name="x", bufs=1))
        wpool = ctx.enter_context(tc.tile_pool(name="w", bufs=1))
        cpool = ctx.enter_context(tc.tile_pool(name="consts", bufs=1))
        opool = ctx.enter_context(tc.tile_pool(name="out", bufs=4))
        spool = ctx.enter_context(tc.tile_pool(name="scratch", bufs=4))
        stats = ctx.enter_context(tc.tile_pool(name="stats", bufs=8))
        psum = ctx.enter_context(tc.tile_pool(name="psum", bufs=8, space="PSUM"))

        # constants + pre-warm scratch (no DMA deps)
        zero_sb = cpool.tile([128, 1], F32)
        nc.vector.memset(zero_sb, 0.0)
        eps_sb = cpool.tile([128, 1], F32)
        nc.vector.memset(eps_sb, EPS)
        dscr = cpool.tile([128, 512], BF16)
        nc.vector.memset(dscr, 0.0)

        # PE pre-warm: keep the PE busy from the preamble to the first weight
        # chunk so the HAM clock gate is at 8/8 for every real matmul.
        dummy = [psum.tile([128, 512], F32, name=f"dummy{i}", tag="yp")
                 for i in range(2)]
        for i in range(N_WARM):
            nc.tensor.matmul(dummy[i % 2], dscr[:, 0:128], dscr,
                             start=True, stop=True)

        w_sb = wpool.tile([128, KB, H], BF16)
        x_sb_all = xpool.tile([128, TB, KB, 128], BF16)
        x_sb = [x_sb_all[:, tt] for tt in range(TB)]
        nw_sb = cpool.tile([128, H], F32) if not nw_ones else None

        def wdma(eng, ib_lo, ib_hi):
            eng.dma_start(out=w_sb[:, ib_lo:ib_hi + 1, :],
                          in_=wt_h[:, ib_lo:ib_hi + 1, :])

        def xdma(eng, tt):
            eng.dma_start(out=x_sb_all[:, tt], in_=xt_h[:, tt])

        def xgroup(eng, lo, hi):
            eng.dma_start(out=x_sb_all[:, lo:hi], in_=xt_h[:, lo:hi])

        # DMA schedule: concurrent queue rows split HBM read bandwidth, so
        # use only the two HWDGE rows early and order each FIFO strictly by
        # consumption time.  Weights interleave across BOTH rows (evens on
        # scalar, odds on sync) so the first token-block pair consumes them
        # at the combined arrival rate; bulk x and outputs ride behind.
        wdma(nc.scalar, 0, 0)
        wdma(nc.scalar, 2, 2)
        wdma(nc.scalar, 4, 4)
        wdma(nc.scalar, 6, 6)
        xdma(nc.sync, 0)
        xdma(nc.sync, 1)
        wdma(nc.sync, 1, 1)
        wdma(nc.sync, 3, 3)
        wdma(nc.sync, 5, 5)
        wdma(nc.sync, 7, 7)
        xdma(nc.sync, 2)
        xgroup(nc.sync, 3, 6)
        xgroup(nc.sync, 6, 10)
        xgroup(nc.sync, 10, 16)
        if not nw_ones:
            nc.gpsimd.dma_start(out=nw_sb, in_=nw_h[:, :])

        def mm_group(tbs, ib_outer):
            """Accumulate y for the token blocks in `tbs` into per-half psum
            tiles.  ib_outer=True consumes each weight chunk for all blocks
            and both halves the moment it lands (weight-arrival order)."""
            yps = {tt: [psum.tile([128, 512], F32, name=f"yp{tt}h{h}", tag="yp")
                        for h in range(2)] for tt in tbs}
            if ib_outer:
                for ib in (0, 2, 1, 4, 3, 6, 5, 7):
                    for tt in tbs:
                        for h in range(2):
                            nc.tensor.matmul(
                                yps[tt][h], x_sb[tt][:, ib, :],
                                w_sb[:, ib, h * 512:(h + 1) * 512],
                                start=(ib == 0), stop=(ib == 7))
            else:
                for tt in tbs:
                    for h in range(2):
                        for ib in range(KB):
                            nc.tensor.matmul(
                                yps[tt][h], x_sb[tt][:, ib, :],
                                w_sb[:, ib, h * 512:(h + 1) * 512],
                                start=(ib == 0), stop=(ib == KB - 1))
            return yps

        def norm_out(tt, yph):
            # sum of squares per half on ACT (hardware accumulator), then
            # rstd = 1/sqrt(mean + eps), then out = (y * rstd) [* norm_w]
            hs = stats.tile([128, 2], F32, name=f"hs{tt}", tag="hs")
            for h in range(2):
                sq = spool.tile([128, 512], BF16, name=f"sq{tt}h{h}", tag="sq")
                nc.scalar.activation(out=sq, in_=yph[h], func=AF.Square,
                                     bias=zero_sb, accum_out=hs[:, h:h + 1])
            ssum = stats.tile([128, 1], F32, name=f"ss{tt}", tag="ss")
            nc.vector.tensor_add(out=ssum, in0=hs[:, 0:1], in1=hs[:, 1:2])
            std = stats.tile([128, 1], F32, name=f"sd{tt}", tag="sd")
            nc.scalar.activation(out=std, in_=ssum, func=AF.Sqrt,
                                 bias=eps_sb, scale=1.0 / H)
            rstd = stats.tile([128, 1], F32, name=f"rs{tt}", tag="rs")
            nc.vector.reciprocal(out=rstd, in_=std)

            o = opool.tile([128, H], BF16, name=f"o{tt}", tag="o")
            for h in range(2):
                sl = slice(h * 512, (h + 1) * 512)
                if nw_ones:
                    # halves on different engines -> they run concurrently
                    if h == 0:
                        nc.scalar.activation(out=o[:, sl], in_=yph[h],
                                             func=AF.Copy, scale=rstd)
                    else:
                        nc.vector.scalar_tensor_tensor(
                            out=o[:, sl], in0=yph[h], scalar=rstd,
                            in1=dscr, op0=OP.mult, op1=OP.bypass)
                else:
                    nc.vector.scalar_tensor_tensor(
                        out=o[:, sl], in0=yph[h], scalar=rstd,
                        in1=nw_sb[:, sl], op0=OP.mult, op1=OP.mult)
            if tt < 14:
                eng = nc.scalar if tt % 2 == 0 else nc.sync
                eng.dma_start(out=out_h[tt * 128:(tt + 1) * 128, :], in_=o)
            else:
                # tail blocks: per-half DMAs on both rings so the last
                # transfer issues the moment its half is ready
                for h in range(2):
                    sl = slice(h * 512, (h + 1) * 512)
                    eng = nc.scalar if (tt + h) % 2 == 0 else nc.sync
                    eng.dma_start(out=out_h[tt * 128:(tt + 1) * 128, sl],
                                  in_=o[:, sl])

        # token-block schedule: first three blocks as one ib-outer group
        # (each arriving weight chunk unlocks 6 warm matmuls), then pairs,
        # then tb15 alone half-major for the shortest tail.
        groups = ([[0, 1]] + [[2, 3], [4, 5], [6, 7], [8, 9], [10, 11],
                  [12, 13]] + [[14], [15]])
        for gi, tbs in enumerate(groups):
            yps = mm_group(tbs, ib_outer=gi == 0)
            for tt in tbs:
                norm_out(tt, yps[tt])

    _legalize_multiwait(nc)
    return nc


def host_prep(x, conv_w, norm_w):
    """Shard + lay out the full inputs into per-core device input maps."""
    bf16 = ml_dtypes.bfloat16

    # Collapse the 20 1x1 convs: W[o,i] = sum_l conv_w[l,o,i] / L
    w = np.asarray(conv_w).sum(axis=0) * (1.0 / L)          # [H(o), H(i)] f32
    # wt[p, ib, o] = W[o, ib*128+p]
    wt = np.ascontiguousarray(
        w.reshape(H, KB, 128).transpose(2, 1, 0).astype(bf16))
    nw = np.ascontiguousarray(np.broadcast_to(
        np.asarray(norm_w, dtype=np.float32), (128, H)))

    x2d = np.asarray(x).reshape(TOK, H)
    xbf = x2d.astype(bf16)

    in_maps = []
    for c in range(N_CORES):
        xc = xbf[c * TPC:(c + 1) * TPC]                      # [TPC, H]
        # xt[p, tt, ib, t] = xc[tt*128+t, ib*128+p]
        xtc = np.ascontiguousarray(
            xc.reshape(TB, 128, KB, 128).transpose(3, 0, 2, 1))
        in_maps.append({"xt": xtc, "wt": wt, "nw": nw})
    return in_maps


def kernel(x, conv_w, norm_w):
    global LAST_RESULTS
    nw_ones = bool(np.all(np.asarray(norm_w) == 1.0))
    if nw_ones not in _BUILT:
        _BUILT[nw_ones] = _build(nw_ones)
    nc = _BUILT[nw_ones]

    x = np.asarray(x)
    out_dtype = x.dtype
    in_maps = host_prep(x, conv_w, norm_w)

    res = run_bass_kernel_spmd(nc, in_maps, core_ids=list(range(N_CORES)))
    LAST_RESULTS = res

    out = np.concatenate([r["out"] for r in res.results], axis=0)
    return out.reshape(B, S, H).astype(out_dtype, copy=False)
